# revision 10
# baseline (speedup 1.0000x reference)
"""DGCNN forward kernel for Trainium2 (Bass/Tile), 8-core data-parallel over graphs.

Full inputs in, full outputs out. Host-side prep (index-only work + layout):
each graph's dense normalized adjacency-transpose M''[u,v] = (count(src=u,
dst=v) + I[u,v]) / (deg_v + 1) is built in f32 and DMA'd in, so the device
runs the 4 GNN layers as dense matmuls (A-matmuls in float32r mode for full
PE rate, small f32 W/bias matmuls, one PSUM copy + one tanh per layer), an
exact pairwise-rank sortpool (is_gt with fused row-sum, is_eq on GPSIMD,
eq*tri + rank fused in one tensor_tensor_reduce), and a bf16 conv head with
host-pretransposed weights.
"""
import sys

sys.path.insert(0, "/opt/trn_rl_repo")

import numpy as np

import concourse.bacc as bacc
import concourse.mybir as mybir
import concourse.tile as tile
from concourse.masks import make_identity

N_CORES = 8
B = 256          # total graphs
NPG = 256        # nodes per graph
F = 128          # input feature dim
EPG = 4096       # edges per graph
K = 30           # sortpool k
D = 97           # total latent dim
LAT = [32, 32, 32, 1]
C1, C2, KW2 = 16, 32, 5
NP2 = 11         # conv2 output positions per graph
f32 = mybir.dt.float32
f32r = mybir.dt.float32r
i32 = mybir.dt.int32
bf16 = mybir.dt.bfloat16
AF = mybir.ActivationFunctionType
OP = mybir.AluOpType


def build_nc(G, debug=False):
    nc = bacc.Bacc("TRN2", target_bir_lowering=False, debug=debug)
    N = G * NPG
    dims = [F] + LAT

    mT = nc.dram_tensor("mT", (N, NPG), f32, kind="ExternalInput")
    nfT = nc.dram_tensor("node_feat", (N, F), f32, kind="ExternalInput")
    rdbT = nc.dram_tensor("rdb", (1, N), f32, kind="ExternalInput")
    Wd = [nc.dram_tensor(f"W{i}", (dims[i], dims[i + 1]), f32, kind="ExternalInput")
          for i in range(4)]
    bd = [nc.dram_tensor(f"b{i}", (1, dims[i + 1]), f32, kind="ExternalInput")
          for i in range(4)]
    c1rT = nc.dram_tensor("c1r", (D, C1), bf16, kind="ExternalInput")
    c1bT = nc.dram_tensor("c1b", (C1, 1), f32, kind="ExternalInput")
    c2rT = nc.dram_tensor("c2r", (C1, KW2 * C2), bf16, kind="ExternalInput")
    c2bT = nc.dram_tensor("c2b", (C2, 1), f32, kind="ExternalInput")
    owT = nc.dram_tensor("ow", (C2, NP2 * 2), bf16, kind="ExternalInput")
    obT = nc.dram_tensor("ob", (1, 2), bf16, kind="ExternalInput")
    outT = nc.dram_tensor("out", (G, 2), f32, kind="ExternalOutput")

    def fr(ap):
        return ap.bitcast(f32r)

    with tile.TileContext(nc) as tc:
        with (
            tc.tile_pool(name="const", bufs=1) as cpool,
            tc.tile_pool(name="big", bufs=1) as bigpool,
        ):
            ident = cpool.tile([128, 128], f32)
            make_identity(nc, ident[:])

            iota_i = cpool.tile([128, 256], i32)
            nc.gpsimd.iota(iota_i[:], pattern=[[1, 256]], base=0, channel_multiplier=0)
            iota30f = cpool.tile([128, K], f32)
            nc.vector.tensor_copy(iota30f[:], iota_i[:, :K])

            # tri[p, c, j] = 1.0 if j < p + 128*c  (strictly-lower mask per chunk)
            tri = cpool.tile([128, 2, 256], f32)
            tmp_i = cpool.tile([128, 256], i32)
            for c in range(2):
                nc.gpsimd.iota(tmp_i[:], pattern=[[1, 256]], base=-128 * c,
                               channel_multiplier=-1)
                nc.vector.tensor_scalar(tri[:, c, :], tmp_i[:], 0, None,
                                        op0=OP.is_lt)

            ones_g = cpool.tile([1, max(G, 2)], bf16)
            nc.vector.memset(ones_g[:], 1.0)

            w_sb = []
            b_sb = []
            for i in range(4):
                wt = cpool.tile([dims[i], dims[i + 1]], f32, tag=f"w{i}", name=f"w{i}")
                nc.sync.dma_start(out=wt[:], in_=Wd[i][:])
                w_sb.append(wt)
                bt = cpool.tile([1, dims[i + 1]], f32, tag=f"b{i}", name=f"b{i}")
                nc.sync.dma_start(out=bt[:], in_=bd[i][:])
                b_sb.append(bt)
            c1r_sb = cpool.tile([D, C1], bf16)
            nc.sync.dma_start(out=c1r_sb[:], in_=c1rT[:])
            c1b_sb = cpool.tile([C1, 1], f32)
            nc.sync.dma_start(out=c1b_sb[:], in_=c1bT[:])
            c2r_sb = cpool.tile([C1, KW2, C2], bf16)
            nc.sync.dma_start(out=c2r_sb[:], in_=c2rT[:].rearrange("p (t c) -> p t c", t=KW2))
            c2b_sb = cpool.tile([C2, 1], f32)
            nc.sync.dma_start(out=c2b_sb[:], in_=c2bT[:])
            ow_sb = cpool.tile([C2, NP2, 2], bf16)
            nc.sync.dma_start(out=ow_sb[:], in_=owT[:].rearrange("p (t c) -> p t c", t=NP2))
            ob_sb = cpool.tile([1, 2], bf16)
            nc.sync.dma_start(out=ob_sb[:], in_=obT[:])

            # bulk inputs
            nf_sb = bigpool.tile([128, 2 * G, F], f32)
            nc.sync.dma_start(out=nf_sb[:], in_=nfT[:].rearrange("(c p) f -> p c f", p=128))
            msb_a = bigpool.tile([128, G, 256], f32)
            msb_b = bigpool.tile([128, G, 256], f32)
            mTr = mT[:].rearrange("(c p) v -> p c v", p=128)
            for q in range(2):
                lo, hi = q * G // 2, (q + 1) * G // 2
                nc.sync.dma_start(out=msb_a[:, lo:hi, :], in_=mTr[:, lo:hi, :])
                nc.sync.dma_start(out=msb_b[:, lo:hi, :], in_=mTr[:, G + lo:G + hi, :])

            def msl(ci):
                return msb_a[:, ci, :] if ci < G else msb_b[:, ci - G, :]
            rd_sb = bigpool.tile([1, N], f32)
            nc.sync.dma_start(out=rd_sb[:], in_=rdbT[:])

            zall = bigpool.tile([128, G, 2, D], f32)
            z4sb = bigpool.tile([128, G, 2], f32)
            spT_all = bigpool.tile([D, G * K], bf16)

            with (
                tc.tile_pool(name="ptp", bufs=3) as ptpool,
                tc.tile_pool(name="spw", bufs=2) as sppool,
                tc.tile_pool(name="ppsum", bufs=2, space="PSUM") as ppsum,
                tc.tile_pool(name="zpsum", bufs=2, space="PSUM") as zpsum,
                tc.tile_pool(name="spsum", bufs=2, space="PSUM") as spsum,
                tc.tile_pool(name="tpsum", bufs=2, space="PSUM") as tpsum,
            ):
                # per-layer PSUM->SBUF copy engine rotation (balance DVE/Act/GPS)
                def copy_pt(li, dst, src):
                    if li == 0:
                        nc.vector.tensor_copy(dst, src)
                    else:
                        nc.scalar.copy(dst, src)

                def layers(g):
                    hc = [nf_sb[:, 2 * g + c, :] for c in range(2)]
                    rowoff = 0
                    for li in range(4):
                        fin, fout = dims[li], dims[li + 1]
                        pT = ppsum.tile([fin, 256], f32, tag="pT", name="pT")
                        for c in range(2):
                            nc.tensor.matmul(out=pT[:], lhsT=hc[c],
                                             rhs=msl(2 * g + c),
                                             start=(c == 0), stop=(c == 1))
                        pT_sb = ptpool.tile([fin, 256], f32, tag=f"ptsb{li}",
                                            name="pT_sb")
                        copy_pt(li, pT_sb[:], pT[:])
                        zc = zpsum.tile([128, 2, fout], f32, tag="zc", name="zc")
                        for c in range(2):
                            nc.tensor.matmul(out=zc[:, c, :],
                                             lhsT=pT_sb[:, c * 128:(c + 1) * 128],
                                             rhs=w_sb[li][:], start=True, stop=False)
                            nc.tensor.matmul(
                                out=zc[:, c, :],
                                lhsT=rd_sb[0:1, g * NPG + c * 128:g * NPG + (c + 1) * 128],
                                rhs=b_sb[li][:], start=False, stop=True)
                        if li == 3:
                            nc.vector.tensor_copy(z4sb[:, g, :], zc[:, :, 0])
                        nc.scalar.activation(zall[:, g, :, rowoff:rowoff + fout],
                                             zc[:, :, :], AF.Tanh)
                        hc = [zall[:, g, c, rowoff:rowoff + fout] for c in range(2)]
                        rowoff += fout

                def sortpool(g):
                    # z4 column chunks -> one [1,256] row via PE transposes,
                    # then broadcast down partitions (GPSIMD reads PSUM).
                    vb = sppool.tile([128, 256], f32, tag="vb")
                    zr = tpsum.tile([1, 2, 128], f32, tag="zr", name="zr")
                    for c in range(2):
                        nc.tensor.transpose(out=zr[:, c, :], in_=z4sb[:, g, c:c + 1],
                                            identity=ident[:])
                    z4row = sppool.tile([1, 256], f32, tag="z4row")
                    nc.vector.tensor_copy(z4row[:], zr[:])
                    nc.gpsimd.partition_broadcast(vb[:], z4row[:])
                    spt = spsum.tile([D, K], f32, tag="spt", name="spt")
                    for c in range(2):
                        r1 = sppool.tile([128, 1], f32, tag="r1", name="r1")
                        gts = sppool.tile([128, 256], f32, tag="gts", name="gts")
                        nc.vector.tensor_scalar(gts[:], vb[:], z4sb[:, g, c:c + 1],
                                                None, op0=OP.is_gt, op1=OP.add,
                                                accum_out=r1[:])
                        eqs = sppool.tile([128, 256], f32, tag="eqs", name="eqs")
                        nc.gpsimd.tensor_scalar(eqs[:], vb[:], z4sb[:, g, c:c + 1],
                                                None, op0=OP.is_equal)
                        em = sppool.tile([128, 256], f32, tag="em", name="em")
                        rank = sppool.tile([128, 1], f32, tag="rank", name="rank")
                        r2 = sppool.tile([128, 1], f32, tag="r2", name="r2")
                        nc.vector.tensor_tensor(out=em[:], in0=eqs[:], in1=tri[:, c, :],
                                                op=OP.mult)
                        nc.vector.tensor_reduce(r2[:], em[:], axis=mybir.AxisListType.X,
                                                op=OP.add)
                        nc.vector.tensor_tensor(out=rank[:], in0=r1[:], in1=r2[:],
                                                op=OP.add)
                        P = sppool.tile([128, K], f32, tag="P", name="P")
                        nc.vector.tensor_scalar(P[:], iota30f[:], rank[:], None,
                                                op0=OP.is_equal)
                        nc.tensor.matmul(out=spt[:], lhsT=zall[:, g, c, :], rhs=P[:],
                                         start=(c == 0), stop=(c == 1))
                    nc.scalar.copy(spT_all[:, g * K:(g + 1) * K], spt[:])

                for g in range(G):
                    layers(g)
                    if g > 0:
                        sortpool(g - 1)
                sortpool(G - 1)

            # ---------------- conv head, batched over graphs ----------------
            with (
                tc.tile_pool(name="head", bufs=1) as hpool,
                tc.tile_pool(name="hpsum", bufs=2, space="PSUM") as hpsum,
            ):
                GK = G * K
                y1 = hpool.tile([C1, GK], bf16)
                half = (GK // 2 + K - 1) // K * K  # split on graph boundary
                for s, e in ((0, half), (half, GK)):
                    y1p = hpsum.tile([C1, max(half, GK - half)], f32, tag="y1p",
                                     name="y1p")
                    nc.tensor.matmul(out=y1p[:, :e - s], lhsT=c1r_sb[:],
                                     rhs=spT_all[:, s:e], start=True, stop=True)
                    nc.scalar.activation(y1[:, s:e], y1p[:, :e - s], AF.Relu,
                                         bias=c1b_sb[:])
                yp = hpool.tile([C1, G * (K // 2)], bf16)
                nc.vector.tensor_reduce(yp[:],
                                        y1[:].rearrange("c (q two) -> c q two", two=2),
                                        axis=mybir.AxisListType.X, op=OP.max)
                yp3 = yp[:].rearrange("c (g q) -> c g q", g=G)
                y2p = hpsum.tile([C2, G * NP2], f32, tag="y2p")
                for t in range(KW2):
                    nc.tensor.matmul(out=y2p[:], lhsT=c2r_sb[:, t, :],
                                     rhs=yp3[:, :, t:t + NP2],
                                     start=(t == 0), stop=(t == KW2 - 1))
                y2 = hpool.tile([C2, G * NP2], bf16)
                nc.scalar.activation(y2[:], y2p[:], AF.Relu, bias=c2b_sb[:])
                y23 = y2[:].rearrange("c (g p) -> c g p", g=G)
                op_ = hpsum.tile([G, 2], f32, tag="op")
                for p in range(NP2):
                    nc.tensor.matmul(out=op_[:], lhsT=y23[:, :, p], rhs=ow_sb[:, p, :],
                                     start=(p == 0), stop=False)
                nc.tensor.matmul(out=op_[:], lhsT=ones_g[:, :G], rhs=ob_sb[:],
                                 start=False, stop=True)
                ores = hpool.tile([G, 2], f32)
                nc.scalar.activation(ores[:], op_[:], AF.Relu)
                nc.sync.dma_start(out=outT[:], in_=ores[:])

    nc.compile()
    return nc


_NC_CACHE = {}


def _get_nc(G):
    if G not in _NC_CACHE:
        _NC_CACHE[G] = build_nc(G)
    return _NC_CACHE[G]


def make_in_maps(inputs, n_cores=N_CORES):
    """Host prep: per-graph dense normalized adjacency (index-only work +
    casts), pre-transposed head weights."""
    import ml_dtypes
    bf = ml_dtypes.bfloat16
    G = B // n_cores
    npc = G * NPG

    src = np.asarray(inputs["src"]).astype(np.int64)
    dst = np.asarray(inputs["dst"]).astype(np.int64)
    degs = np.asarray(inputs["degs"]).astype(np.float32)
    rd = (1.0 / (degs + 1.0)).astype(np.float32)
    nf = np.ascontiguousarray(np.asarray(inputs["node_feat"], np.float32))

    # dense M''^T per graph: M[u, v] = (count(src=u,dst=v) + I[u,v]) * rd[v]
    srcl = src % NPG
    dstl = dst % NPG
    gid = src // NPG
    flat = gid * (NPG * NPG) + srcl * NPG + dstl
    cnt = np.bincount(flat, minlength=B * NPG * NPG).astype(np.float32)
    cnt = cnt.reshape(B, NPG, NPG)
    idx = np.arange(NPG)
    cnt[:, idx, idx] += 1.0
    cnt *= rd.reshape(B, 1, NPG)
    mT_all = cnt.reshape(B * NPG, NPG)

    c1r = np.asarray(inputs["conv1_w"], np.float32).reshape(C1, D).T.copy()
    c2r = np.asarray(inputs["conv2_w"], np.float32).transpose(1, 2, 0).reshape(
        C1, KW2 * C2).copy()
    ow = np.asarray(inputs["out_w"], np.float32).reshape(C2, NP2, 2).reshape(
        C2, NP2 * 2).copy()

    in_maps = []
    for c in range(n_cores):
        m = {
            "mT": np.ascontiguousarray(mT_all[c * npc:(c + 1) * npc]),
            "node_feat": np.ascontiguousarray(nf[c * npc:(c + 1) * npc]),
            "rdb": rd[c * npc:(c + 1) * npc].reshape(1, npc).copy(),
            "c1r": c1r.astype(bf),
            "c1b": np.asarray(inputs["conv1_b"], np.float32).reshape(C1, 1),
            "c2r": c2r.astype(bf),
            "c2b": np.asarray(inputs["conv2_b"], np.float32).reshape(C2, 1),
            "ow": ow.astype(bf),
            "ob": np.asarray(inputs["out_b"], np.float32).reshape(1, 2).astype(bf),
        }
        for i in range(4):
            m[f"W{i}"] = np.ascontiguousarray(np.asarray(inputs[f"W{i}"], np.float32))
            m[f"b{i}"] = np.asarray(inputs[f"b{i}"], np.float32).reshape(1, LAT[i])
        in_maps.append(m)
    return in_maps


def kernel(**inputs):
    from concourse import bass_utils
    inputs = {k: np.asarray(v) for k, v in inputs.items()}
    nc = _get_nc(B // N_CORES)
    in_maps = make_in_maps(inputs)
    res = bass_utils.run_bass_kernel_spmd(nc, in_maps, core_ids=list(range(N_CORES)))
    return np.concatenate([np.asarray(r["out"], np.float32) for r in res.results],
                          axis=0)


if __name__ == "__main__":
    nc = build_nc(2)
    print("built ok")


# revision 13
# speedup vs baseline: 1.4503x; 1.4503x over previous
"""DGCNN forward kernel for Trainium2 (Bass/Tile), 8-core data-parallel over graphs.

Full inputs in, full outputs out. Host-side prep (index-only work + layout):
each graph's dense normalized adjacency-transpose M''[u,v] = (count(src=u,
dst=v) + I[u,v]) / (deg_v + 1) is built in f32 and DMA'd in, so the device
runs the 4 GNN layers as dense matmuls (A-matmuls in float32r mode for full
PE rate, small f32 W/bias matmuls, one PSUM copy + one tanh per layer), an
exact pairwise-rank sortpool (is_gt with fused row-sum, is_eq on GPSIMD,
eq*tri + rank fused in one tensor_tensor_reduce), and a bf16 conv head with
host-pretransposed weights.
"""
import sys

sys.path.insert(0, "/opt/trn_rl_repo")

import numpy as np

import concourse.bacc as bacc
import concourse.mybir as mybir
import concourse.tile as tile
from concourse.masks import make_identity

N_CORES = 8
B = 256          # total graphs
NPG = 256        # nodes per graph
F = 128          # input feature dim
EPG = 4096       # edges per graph
K = 30           # sortpool k
D = 97           # total latent dim
LAT = [32, 32, 32, 1]
C1, C2, KW2 = 16, 32, 5
NP2 = 11         # conv2 output positions per graph
f32 = mybir.dt.float32
f32r = mybir.dt.float32r
i32 = mybir.dt.int32
bf16 = mybir.dt.bfloat16
AF = mybir.ActivationFunctionType
OP = mybir.AluOpType


def build_nc(G, debug=False):
    nc = bacc.Bacc("TRN2", target_bir_lowering=False, debug=debug)
    N = G * NPG
    dims = [F] + LAT

    mT = nc.dram_tensor("mT", (N, NPG), f32, kind="ExternalInput")
    nfT = nc.dram_tensor("node_feat", (N, F), f32, kind="ExternalInput")
    rdbT = nc.dram_tensor("rdb", (1, N), f32, kind="ExternalInput")
    Wd = [nc.dram_tensor(f"W{i}", (dims[i], dims[i + 1]), f32, kind="ExternalInput")
          for i in range(4)]
    bd = [nc.dram_tensor(f"b{i}", (1, dims[i + 1]), f32, kind="ExternalInput")
          for i in range(4)]
    c1rT = nc.dram_tensor("c1r", (D, C1), bf16, kind="ExternalInput")
    c1bT = nc.dram_tensor("c1b", (C1, 1), f32, kind="ExternalInput")
    c2rT = nc.dram_tensor("c2r", (C1, KW2 * C2), bf16, kind="ExternalInput")
    c2bT = nc.dram_tensor("c2b", (C2, 1), f32, kind="ExternalInput")
    owT = nc.dram_tensor("ow", (C2, NP2 * 2), bf16, kind="ExternalInput")
    obT = nc.dram_tensor("ob", (1, 2), bf16, kind="ExternalInput")
    outT = nc.dram_tensor("out", (G, 2), f32, kind="ExternalOutput")

    def fr(ap):
        return ap.bitcast(f32r)

    with tile.TileContext(nc) as tc:
        with (
            tc.tile_pool(name="const", bufs=1) as cpool,
            tc.tile_pool(name="big", bufs=1) as bigpool,
        ):
            ident = cpool.tile([128, 128], f32)
            make_identity(nc, ident[:])

            iota_i = cpool.tile([128, 256], i32)
            nc.gpsimd.iota(iota_i[:], pattern=[[1, 256]], base=0, channel_multiplier=0)
            iota30f = cpool.tile([128, K], f32)
            nc.vector.tensor_copy(iota30f[:], iota_i[:, :K])

            # tri[p, c, j] = 1.0 if j < p + 128*c  (strictly-lower mask per chunk)
            tri = cpool.tile([128, 2, 256], f32)
            tmp_i = cpool.tile([128, 256], i32)
            for c in range(2):
                nc.gpsimd.iota(tmp_i[:], pattern=[[1, 256]], base=-128 * c,
                               channel_multiplier=-1)
                nc.vector.tensor_scalar(tri[:, c, :], tmp_i[:], 0, None,
                                        op0=OP.is_lt)

            ones_g = cpool.tile([1, max(G, 2)], bf16)
            nc.vector.memset(ones_g[:], 1.0)

            w_sb = []
            b_sb = []
            for i in range(4):
                wt = cpool.tile([dims[i], dims[i + 1]], f32, tag=f"w{i}", name=f"w{i}")
                nc.sync.dma_start(out=wt[:], in_=Wd[i][:])
                w_sb.append(wt)
                bt = cpool.tile([1, dims[i + 1]], f32, tag=f"b{i}", name=f"b{i}")
                nc.sync.dma_start(out=bt[:], in_=bd[i][:])
                b_sb.append(bt)
            c1r_sb = cpool.tile([D, C1], bf16)
            nc.sync.dma_start(out=c1r_sb[:], in_=c1rT[:])
            c1b_sb = cpool.tile([C1, 1], f32)
            nc.sync.dma_start(out=c1b_sb[:], in_=c1bT[:])
            c2r_sb = cpool.tile([C1, KW2, C2], bf16)
            nc.sync.dma_start(out=c2r_sb[:], in_=c2rT[:].rearrange("p (t c) -> p t c", t=KW2))
            c2b_sb = cpool.tile([C2, 1], f32)
            nc.sync.dma_start(out=c2b_sb[:], in_=c2bT[:])
            ow_sb = cpool.tile([C2, NP2, 2], bf16)
            nc.sync.dma_start(out=ow_sb[:], in_=owT[:].rearrange("p (t c) -> p t c", t=NP2))
            ob_sb = cpool.tile([1, 2], bf16)
            nc.sync.dma_start(out=ob_sb[:], in_=obT[:])

            # bulk inputs
            nf_sb = bigpool.tile([128, 2 * G, F], f32)
            nc.sync.dma_start(out=nf_sb[:], in_=nfT[:].rearrange("(c p) f -> p c f", p=128))
            msb_a = bigpool.tile([128, G, 256], f32)
            msb_b = bigpool.tile([128, G, 256], f32)
            mTr = mT[:].rearrange("(c p) v -> p c v", p=128)
            for q in range(2):
                lo, hi = q * G // 2, (q + 1) * G // 2
                nc.sync.dma_start(out=msb_a[:, lo:hi, :], in_=mTr[:, lo:hi, :])
                nc.sync.dma_start(out=msb_b[:, lo:hi, :], in_=mTr[:, G + lo:G + hi, :])

            def msl(ci):
                return msb_a[:, ci, :] if ci < G else msb_b[:, ci - G, :]
            rd_sb = bigpool.tile([1, N], f32)
            nc.sync.dma_start(out=rd_sb[:], in_=rdbT[:])

            zall = bigpool.tile([128, G, 2, D], f32)
            z4sb = bigpool.tile([128, G, 2], f32)
            spT_all = bigpool.tile([D, G * K], bf16)

            with (
                tc.tile_pool(name="ptp", bufs=3) as ptpool,
                tc.tile_pool(name="spw", bufs=2) as sppool,
                tc.tile_pool(name="ppsum", bufs=3, space="PSUM") as ppsum,
                tc.tile_pool(name="zpsum", bufs=1, space="PSUM") as zpsum,
                tc.tile_pool(name="spsum", bufs=2, space="PSUM") as spsum,
                tc.tile_pool(name="tpsum", bufs=1, space="PSUM") as tpsum,
            ):
                # per-layer PSUM->SBUF copy engine rotation (balance DVE/Act/GPS)
                def copy_pt(li, dst, src):
                    if li == 0:
                        nc.vector.tensor_copy(dst, src)
                    else:
                        nc.scalar.copy(dst, src)

                offs = [0, 32, 64, 96]

                def layer(g, li):
                    fin, fout = dims[li], dims[li + 1]
                    rowoff = offs[li]
                    if li == 0:
                        hc = [nf_sb[:, 2 * g + c, :] for c in range(2)]
                    else:
                        po = offs[li - 1]
                        hc = [zall[:, g, c, po:po + fin] for c in range(2)]
                    pT = ppsum.tile([fin, 256], f32, tag="pT", name="pT")
                    for c in range(2):
                        nc.tensor.matmul(out=pT[:], lhsT=hc[c],
                                         rhs=msl(2 * g + c),
                                         start=(c == 0), stop=(c == 1))
                    pT_sb = ptpool.tile([fin, 256], f32, tag=f"ptsb{li}",
                                        name="pT_sb")
                    copy_pt(li, pT_sb[:], pT[:])
                    zc = zpsum.tile([128, 2, fout], f32, tag=f"zc{li % 2}", name="zc")
                    for c in range(2):
                        nc.tensor.matmul(out=zc[:, c, :],
                                         lhsT=pT_sb[:, c * 128:(c + 1) * 128],
                                         rhs=w_sb[li][:], start=True, stop=False)
                        nc.tensor.matmul(
                            out=zc[:, c, :],
                            lhsT=rd_sb[0:1, g * NPG + c * 128:g * NPG + (c + 1) * 128],
                            rhs=b_sb[li][:], start=False, stop=True)
                    if li == 3:
                        nc.vector.tensor_copy(z4sb[:, g, :], zc[:, :, 0])
                    nc.scalar.activation(zall[:, g, :, rowoff:rowoff + fout],
                                         zc[:, :, :], AF.Tanh)

                def sortpool(g):
                    vb = sppool.tile([128, 256], f32, tag="vb")
                    zr = tpsum.tile([1, 2, 128], f32, tag="zr", name="zr")
                    for c in range(2):
                        nc.tensor.transpose(out=zr[:, c, :], in_=z4sb[:, g, c:c + 1],
                                            identity=ident[:])
                    z4row = sppool.tile([1, 256], f32, tag="z4row")
                    nc.vector.tensor_copy(z4row[:], zr[:])
                    nc.gpsimd.partition_broadcast(vb[:], z4row[:])
                    spt = spsum.tile([D, K], f32, tag="spt", name="spt")
                    for c in range(2):
                        r1 = sppool.tile([128, 1], f32, tag="r1", name="r1")
                        gts = sppool.tile([128, 256], f32, tag="gts", name="gts")
                        nc.vector.tensor_scalar(gts[:], vb[:], z4sb[:, g, c:c + 1],
                                                None, op0=OP.is_gt, op1=OP.add,
                                                accum_out=r1[:])
                        eqs = sppool.tile([128, 256], f32, tag="eqs", name="eqs")
                        nc.gpsimd.tensor_scalar(eqs[:], vb[:], z4sb[:, g, c:c + 1],
                                                None, op0=OP.is_equal)
                        em = sppool.tile([128, 256], f32, tag="em", name="em")
                        rank = sppool.tile([128, 1], f32, tag="rank", name="rank")
                        r2 = sppool.tile([128, 1], f32, tag="r2", name="r2")
                        nc.vector.tensor_tensor(out=em[:], in0=eqs[:], in1=tri[:, c, :],
                                                op=OP.mult)
                        nc.vector.tensor_reduce(r2[:], em[:], axis=mybir.AxisListType.X,
                                                op=OP.add)
                        nc.vector.tensor_tensor(out=rank[:], in0=r1[:], in1=r2[:],
                                                op=OP.add)
                        P = sppool.tile([128, K], f32, tag="P", name="P")
                        nc.vector.tensor_scalar(P[:], iota30f[:], rank[:], None,
                                                op0=OP.is_equal)
                        nc.tensor.matmul(out=spt[:], lhsT=zall[:, g, c, :], rhs=P[:],
                                         start=(c == 0), stop=(c == 1))
                    nc.scalar.copy(spT_all[:, g * K:(g + 1) * K], spt[:])

                for gg in range(G + 4):
                    for li in range(4):
                        g = gg - li
                        if 0 <= g < G:
                            layer(g, li)
                    if gg >= 4:
                        sortpool(gg - 4)

            # ---------------- conv head, batched over graphs ----------------
            with (
                tc.tile_pool(name="head", bufs=1) as hpool,
                tc.tile_pool(name="hpsum", bufs=2, space="PSUM") as hpsum,
            ):
                GK = G * K
                y1 = hpool.tile([C1, GK], bf16)
                half = (GK // 2 + K - 1) // K * K  # split on graph boundary
                for s, e in ((0, half), (half, GK)):
                    y1p = hpsum.tile([C1, max(half, GK - half)], f32, tag="y1p",
                                     name="y1p")
                    nc.tensor.matmul(out=y1p[:, :e - s], lhsT=c1r_sb[:],
                                     rhs=spT_all[:, s:e], start=True, stop=True)
                    nc.scalar.activation(y1[:, s:e], y1p[:, :e - s], AF.Relu,
                                         bias=c1b_sb[:])
                yp = hpool.tile([C1, G * (K // 2)], bf16)
                nc.vector.tensor_reduce(yp[:],
                                        y1[:].rearrange("c (q two) -> c q two", two=2),
                                        axis=mybir.AxisListType.X, op=OP.max)
                yp3 = yp[:].rearrange("c (g q) -> c g q", g=G)
                y2p = hpsum.tile([C2, G * NP2], f32, tag="y2p")
                for t in range(KW2):
                    nc.tensor.matmul(out=y2p[:], lhsT=c2r_sb[:, t, :],
                                     rhs=yp3[:, :, t:t + NP2],
                                     start=(t == 0), stop=(t == KW2 - 1))
                y2 = hpool.tile([C2, G * NP2], bf16)
                nc.scalar.activation(y2[:], y2p[:], AF.Relu, bias=c2b_sb[:])
                y23 = y2[:].rearrange("c (g p) -> c g p", g=G)
                op_ = hpsum.tile([G, 2], f32, tag="op")
                for p in range(NP2):
                    nc.tensor.matmul(out=op_[:], lhsT=y23[:, :, p], rhs=ow_sb[:, p, :],
                                     start=(p == 0), stop=False)
                nc.tensor.matmul(out=op_[:], lhsT=ones_g[:, :G], rhs=ob_sb[:],
                                 start=False, stop=True)
                ores = hpool.tile([G, 2], f32)
                nc.scalar.activation(ores[:], op_[:], AF.Relu)
                nc.sync.dma_start(out=outT[:], in_=ores[:])

    nc.compile()
    return nc


_NC_CACHE = {}


def _get_nc(G):
    if G not in _NC_CACHE:
        _NC_CACHE[G] = build_nc(G)
    return _NC_CACHE[G]


def make_in_maps(inputs, n_cores=N_CORES):
    """Host prep: per-graph dense normalized adjacency (index-only work +
    casts), pre-transposed head weights."""
    import ml_dtypes
    bf = ml_dtypes.bfloat16
    G = B // n_cores
    npc = G * NPG

    src = np.asarray(inputs["src"]).astype(np.int64)
    dst = np.asarray(inputs["dst"]).astype(np.int64)
    degs = np.asarray(inputs["degs"]).astype(np.float32)
    rd = (1.0 / (degs + 1.0)).astype(np.float32)
    nf = np.ascontiguousarray(np.asarray(inputs["node_feat"], np.float32))

    # dense M''^T per graph: M[u, v] = (count(src=u,dst=v) + I[u,v]) * rd[v]
    srcl = src % NPG
    dstl = dst % NPG
    gid = src // NPG
    flat = gid * (NPG * NPG) + srcl * NPG + dstl
    cnt = np.bincount(flat, minlength=B * NPG * NPG).astype(np.float32)
    cnt = cnt.reshape(B, NPG, NPG)
    idx = np.arange(NPG)
    cnt[:, idx, idx] += 1.0
    cnt *= rd.reshape(B, 1, NPG)
    mT_all = cnt.reshape(B * NPG, NPG)

    c1r = np.asarray(inputs["conv1_w"], np.float32).reshape(C1, D).T.copy()
    c2r = np.asarray(inputs["conv2_w"], np.float32).transpose(1, 2, 0).reshape(
        C1, KW2 * C2).copy()
    ow = np.asarray(inputs["out_w"], np.float32).reshape(C2, NP2, 2).reshape(
        C2, NP2 * 2).copy()

    in_maps = []
    for c in range(n_cores):
        m = {
            "mT": np.ascontiguousarray(mT_all[c * npc:(c + 1) * npc]),
            "node_feat": np.ascontiguousarray(nf[c * npc:(c + 1) * npc]),
            "rdb": rd[c * npc:(c + 1) * npc].reshape(1, npc).copy(),
            "c1r": c1r.astype(bf),
            "c1b": np.asarray(inputs["conv1_b"], np.float32).reshape(C1, 1),
            "c2r": c2r.astype(bf),
            "c2b": np.asarray(inputs["conv2_b"], np.float32).reshape(C2, 1),
            "ow": ow.astype(bf),
            "ob": np.asarray(inputs["out_b"], np.float32).reshape(1, 2).astype(bf),
        }
        for i in range(4):
            m[f"W{i}"] = np.ascontiguousarray(np.asarray(inputs[f"W{i}"], np.float32))
            m[f"b{i}"] = np.asarray(inputs[f"b{i}"], np.float32).reshape(1, LAT[i])
        in_maps.append(m)
    return in_maps


def kernel(**inputs):
    from concourse import bass_utils
    inputs = {k: np.asarray(v) for k, v in inputs.items()}
    nc = _get_nc(B // N_CORES)
    in_maps = make_in_maps(inputs)
    res = bass_utils.run_bass_kernel_spmd(nc, in_maps, core_ids=list(range(N_CORES)))
    return np.concatenate([np.asarray(r["out"], np.float32) for r in res.results],
                          axis=0)


if __name__ == "__main__":
    nc = build_nc(2)
    print("built ok")


# revision 16
# speedup vs baseline: 1.4828x; 1.0224x over previous
"""DGCNN forward kernel for Trainium2 (Bass/Tile), 8-core data-parallel over graphs.

Full inputs in, full outputs out. Host-side prep (index-only work + layout):
each graph's dense normalized adjacency-transpose M''[u,v] = (count(src=u,
dst=v) + I[u,v]) / (deg_v + 1) is built in f32 and DMA'd in, so the device
runs the 4 GNN layers as dense matmuls (A-matmuls in float32r mode for full
PE rate, small f32 W/bias matmuls, one PSUM copy + one tanh per layer), an
exact pairwise-rank sortpool (is_gt with fused row-sum, is_eq on GPSIMD,
eq*tri + rank fused in one tensor_tensor_reduce), and a bf16 conv head with
host-pretransposed weights.
"""
import sys

sys.path.insert(0, "/opt/trn_rl_repo")

import numpy as np

import concourse.bacc as bacc
import concourse.mybir as mybir
import concourse.tile as tile
from concourse.masks import make_identity

N_CORES = 8
B = 256          # total graphs
NPG = 256        # nodes per graph
F = 128          # input feature dim
EPG = 4096       # edges per graph
K = 30           # sortpool k
D = 97           # total latent dim
LAT = [32, 32, 32, 1]
C1, C2, KW2 = 16, 32, 5
NP2 = 11         # conv2 output positions per graph
f32 = mybir.dt.float32
f32r = mybir.dt.float32r
i32 = mybir.dt.int32
bf16 = mybir.dt.bfloat16
AF = mybir.ActivationFunctionType
OP = mybir.AluOpType


def build_nc(G, debug=False):
    nc = bacc.Bacc("TRN2", target_bir_lowering=False, debug=debug)
    N = G * NPG
    dims = [F] + LAT

    mT = nc.dram_tensor("mT", (N, NPG), f32, kind="ExternalInput")
    nfT = nc.dram_tensor("node_feat", (N, F), f32, kind="ExternalInput")
    rdbT = nc.dram_tensor("rdb", (1, N), f32, kind="ExternalInput")
    Wd = [nc.dram_tensor(f"W{i}", (dims[i], dims[i + 1]), f32, kind="ExternalInput")
          for i in range(4)]
    bd = [nc.dram_tensor(f"b{i}", (1, dims[i + 1]), f32, kind="ExternalInput")
          for i in range(4)]
    c1rT = nc.dram_tensor("c1r", (D, C1), bf16, kind="ExternalInput")
    c1bT = nc.dram_tensor("c1b", (C1, 1), f32, kind="ExternalInput")
    c2rT = nc.dram_tensor("c2r", (C1, KW2 * C2), bf16, kind="ExternalInput")
    c2bT = nc.dram_tensor("c2b", (C2, 1), f32, kind="ExternalInput")
    owT = nc.dram_tensor("ow", (C2, NP2 * 2), bf16, kind="ExternalInput")
    obT = nc.dram_tensor("ob", (1, 2), bf16, kind="ExternalInput")
    outT = nc.dram_tensor("out", (G, 2), f32, kind="ExternalOutput")

    def fr(ap):
        return ap.bitcast(f32r)

    with tile.TileContext(nc) as tc:
        with (
            tc.tile_pool(name="const", bufs=1) as cpool,
            tc.tile_pool(name="big", bufs=1) as bigpool,
        ):
            ident = cpool.tile([128, 128], f32)
            make_identity(nc, ident[:])

            iota_i = cpool.tile([128, 256], i32)
            nc.gpsimd.iota(iota_i[:], pattern=[[1, 256]], base=0, channel_multiplier=0)
            iota30f = cpool.tile([128, K], f32)
            nc.vector.tensor_copy(iota30f[:], iota_i[:, :K])

            # antitri[p, c, j] = 1e30 if j >= p + 128*c else 0 (mask for the
            # strict-lower tie count: adding it to the value row makes every
            # j >= p position unequal to any finite z4 value)
            antitri = cpool.tile([128, 2, 256], f32)
            tmp_i = cpool.tile([128, 256], i32)
            for c in range(2):
                nc.gpsimd.iota(tmp_i[:], pattern=[[1, 256]], base=-128 * c,
                               channel_multiplier=-1)
                nc.vector.tensor_scalar(antitri[:, c, :], tmp_i[:], 0, 1e30,
                                        op0=OP.is_ge, op1=OP.mult)

            ones_g = cpool.tile([1, max(G, 2)], bf16)
            nc.vector.memset(ones_g[:], 1.0)

            w_sb = []
            b_sb = []
            for i in range(4):
                wt = cpool.tile([dims[i], dims[i + 1]], f32, tag=f"w{i}", name=f"w{i}")
                nc.sync.dma_start(out=wt[:], in_=Wd[i][:])
                w_sb.append(wt)
                btf = cpool.tile([1, dims[i + 1]], f32, tag=f"bf{i}", name=f"btf")
                nc.sync.dma_start(out=btf[:], in_=bd[i][:])
                bt = cpool.tile([1, dims[i + 1]], bf16, tag=f"b{i}", name=f"bt")
                nc.vector.tensor_copy(bt[:], btf[:])
                b_sb.append(bt)
            c1r_sb = cpool.tile([D, C1], bf16)
            nc.sync.dma_start(out=c1r_sb[:], in_=c1rT[:])
            c1b_sb = cpool.tile([C1, 1], f32)
            nc.sync.dma_start(out=c1b_sb[:], in_=c1bT[:])
            c2r_sb = cpool.tile([C1, KW2, C2], bf16)
            nc.sync.dma_start(out=c2r_sb[:], in_=c2rT[:].rearrange("p (t c) -> p t c", t=KW2))
            c2b_sb = cpool.tile([C2, 1], f32)
            nc.sync.dma_start(out=c2b_sb[:], in_=c2bT[:])
            ow_sb = cpool.tile([C2, NP2, 2], bf16)
            nc.sync.dma_start(out=ow_sb[:], in_=owT[:].rearrange("p (t c) -> p t c", t=NP2))
            ob_sb = cpool.tile([1, 2], bf16)
            nc.sync.dma_start(out=ob_sb[:], in_=obT[:])

            # bulk inputs
            nf_sb = bigpool.tile([128, 2 * G, F], f32)
            nc.sync.dma_start(out=nf_sb[:], in_=nfT[:].rearrange("(c p) f -> p c f", p=128))
            msb_a = bigpool.tile([128, G, 256], f32)
            msb_b = bigpool.tile([128, G, 256], f32)
            mTr = mT[:].rearrange("(c p) v -> p c v", p=128)
            for q in range(2):
                lo, hi = q * G // 2, (q + 1) * G // 2
                nc.sync.dma_start(out=msb_a[:, lo:hi, :], in_=mTr[:, lo:hi, :])
                nc.sync.dma_start(out=msb_b[:, lo:hi, :], in_=mTr[:, G + lo:G + hi, :])

            def msl(ci):
                return msb_a[:, ci, :] if ci < G else msb_b[:, ci - G, :]
            rd_sb = bigpool.tile([1, N], f32)
            nc.sync.dma_start(out=rd_sb[:], in_=rdbT[:])
            rdb_sb = bigpool.tile([1, N], bf16)
            nc.vector.tensor_copy(rdb_sb[:], rd_sb[:])

            zall = bigpool.tile([128, G, 2, D], f32)
            z4sb = bigpool.tile([128, G, 2], f32)
            spT_all = bigpool.tile([D, G * K], bf16)

            with (
                tc.tile_pool(name="ptp", bufs=3) as ptpool,
                tc.tile_pool(name="spw", bufs=2) as sppool,
                tc.tile_pool(name="ppsum", bufs=4, space="PSUM") as ppsum,
                tc.tile_pool(name="zpsum", bufs=1, space="PSUM") as zpsum,
                tc.tile_pool(name="spsum", bufs=2, space="PSUM") as spsum,
                tc.tile_pool(name="tpsum", bufs=1, space="PSUM") as tpsum,
            ):
                # per-layer PSUM->SBUF copy engine rotation (balance DVE/Act/GPS)
                def copy_pt(li, dst, src):
                    if li in (0, 2):
                        nc.vector.tensor_copy(dst, src)
                    else:
                        nc.scalar.copy(dst, src)

                offs = [0, 32, 64, 96]
                zc_all4 = zpsum.tile([128, 4, 2, 32], f32, tag="zcs", name="zc_all4")

                def layer(g, li):
                    fin, fout = dims[li], dims[li + 1]
                    rowoff = offs[li]
                    if li == 0:
                        hc = [nf_sb[:, 2 * g + c, :] for c in range(2)]
                    else:
                        po = offs[li - 1]
                        hc = [zall[:, g, c, po:po + fin] for c in range(2)]
                    pT = ppsum.tile([fin, 256], f32, tag="pT", name="pT")
                    for c in range(2):
                        nc.tensor.matmul(out=pT[:], lhsT=hc[c],
                                         rhs=msl(2 * g + c),
                                         start=(c == 0), stop=(c == 1))
                    pT_sb = ptpool.tile([fin, 256], f32, tag=f"ptsb{li}",
                                        name="pT_sb")
                    copy_pt(li, pT_sb[:], pT[:])
                    zc = zc_all4[:, li, :, :fout]
                    for c in range(2):
                        nc.tensor.matmul(out=zc[:, c, :],
                                         lhsT=pT_sb[:, c * 128:(c + 1) * 128],
                                         rhs=w_sb[li][:], start=True, stop=False)
                        nc.tensor.matmul(
                            out=zc[:, c, :],
                            lhsT=rdb_sb[0:1, g * NPG + c * 128:g * NPG + (c + 1) * 128],
                            rhs=b_sb[li][:], start=False, stop=True)
                    if li == 3:
                        nc.vector.tensor_copy(z4sb[:, g, :], zc[:, :, 0])
                    nc.scalar.activation(zall[:, g, :, rowoff:rowoff + fout],
                                         zc[:, :, :], AF.Tanh)

                def sortpool(g):
                    vb = sppool.tile([128, 256], f32, tag="vb")
                    zr = tpsum.tile([1, 2, 128], f32, tag="zr", name="zr")
                    for c in range(2):
                        nc.tensor.transpose(out=zr[:, c, :], in_=z4sb[:, g, c:c + 1],
                                            identity=ident[:])
                    z4row = sppool.tile([1, 256], f32, tag="z4row")
                    nc.scalar.copy(z4row[:], zr[:])
                    nc.gpsimd.partition_broadcast(vb[:], z4row[:])
                    spt = spsum.tile([D, K], f32, tag="spt", name="spt")
                    for c in range(2):
                        vbm = sppool.tile([128, 256], f32, tag="vbm", name="vbm")
                        nc.gpsimd.tensor_tensor(out=vbm[:], in0=vb[:],
                                                in1=antitri[:, c, :], op=OP.add)
                        r1 = sppool.tile([128, 1], f32, tag="r1", name="r1")
                        gts = sppool.tile([128, 256], f32, tag="gts", name="gts")
                        nc.vector.tensor_scalar(gts[:], vb[:], z4sb[:, g, c:c + 1],
                                                None, op0=OP.is_gt, op1=OP.add,
                                                accum_out=r1[:])
                        eqs = sppool.tile([128, 256], f32, tag="eqs", name="eqs")
                        r2 = sppool.tile([128, 1], f32, tag="r2", name="r2")
                        nc.vector.tensor_scalar(eqs[:], vbm[:], z4sb[:, g, c:c + 1],
                                                None, op0=OP.is_equal, op1=OP.add,
                                                accum_out=r2[:])
                        rank = sppool.tile([128, 1], f32, tag="rank", name="rank")
                        nc.vector.tensor_tensor(out=rank[:], in0=r1[:], in1=r2[:],
                                                op=OP.add)
                        P = sppool.tile([128, K], f32, tag="P", name="P")
                        nc.vector.tensor_scalar(P[:], iota30f[:], rank[:], None,
                                                op0=OP.is_equal)
                        nc.tensor.matmul(out=spt[:], lhsT=zall[:, g, c, :], rhs=P[:],
                                         start=(c == 0), stop=(c == 1))
                    nc.scalar.copy(spT_all[:, g * K:(g + 1) * K], spt[:])

                for gg in range(G + 4):
                    for li in range(4):
                        g = gg - li
                        if 0 <= g < G:
                            layer(g, li)
                    if gg >= 4:
                        sortpool(gg - 4)

            # ---------------- conv head, batched over graphs ----------------
            with (
                tc.tile_pool(name="head", bufs=1) as hpool,
                tc.tile_pool(name="hpsum", bufs=2, space="PSUM") as hpsum,
            ):
                GK = G * K
                y1 = hpool.tile([C1, GK], bf16)
                half = (GK // 2 + K - 1) // K * K  # split on graph boundary
                for s, e in ((0, half), (half, GK)):
                    y1p = hpsum.tile([C1, max(half, GK - half)], f32, tag="y1p",
                                     name="y1p")
                    nc.tensor.matmul(out=y1p[:, :e - s], lhsT=c1r_sb[:],
                                     rhs=spT_all[:, s:e], start=True, stop=True)
                    nc.scalar.activation(y1[:, s:e], y1p[:, :e - s], AF.Relu,
                                         bias=c1b_sb[:])
                yp = hpool.tile([C1, G * (K // 2)], bf16)
                nc.vector.tensor_reduce(yp[:],
                                        y1[:].rearrange("c (q two) -> c q two", two=2),
                                        axis=mybir.AxisListType.X, op=OP.max)
                yp3 = yp[:].rearrange("c (g q) -> c g q", g=G)
                y2p = hpsum.tile([C2, G * NP2], f32, tag="y2p")
                for t in range(KW2):
                    nc.tensor.matmul(out=y2p[:], lhsT=c2r_sb[:, t, :],
                                     rhs=yp3[:, :, t:t + NP2],
                                     start=(t == 0), stop=(t == KW2 - 1))
                y2 = hpool.tile([C2, G * NP2], bf16)
                nc.scalar.activation(y2[:], y2p[:], AF.Relu, bias=c2b_sb[:])
                y23 = y2[:].rearrange("c (g p) -> c g p", g=G)
                op_ = hpsum.tile([G, 2], f32, tag="op")
                for p in range(NP2):
                    nc.tensor.matmul(out=op_[:], lhsT=y23[:, :, p], rhs=ow_sb[:, p, :],
                                     start=(p == 0), stop=False)
                nc.tensor.matmul(out=op_[:], lhsT=ones_g[:, :G], rhs=ob_sb[:],
                                 start=False, stop=True)
                ores = hpool.tile([G, 2], f32)
                nc.scalar.activation(ores[:], op_[:], AF.Relu)
                nc.sync.dma_start(out=outT[:], in_=ores[:])

    nc.compile()
    return nc


_NC_CACHE = {}


def _get_nc(G):
    if G not in _NC_CACHE:
        _NC_CACHE[G] = build_nc(G)
    return _NC_CACHE[G]


def make_in_maps(inputs, n_cores=N_CORES):
    """Host prep: per-graph dense normalized adjacency (index-only work +
    casts), pre-transposed head weights."""
    import ml_dtypes
    bf = ml_dtypes.bfloat16
    G = B // n_cores
    npc = G * NPG

    src = np.asarray(inputs["src"]).astype(np.int64)
    dst = np.asarray(inputs["dst"]).astype(np.int64)
    degs = np.asarray(inputs["degs"]).astype(np.float32)
    rd = (1.0 / (degs + 1.0)).astype(np.float32)
    nf = np.ascontiguousarray(np.asarray(inputs["node_feat"], np.float32))

    # dense M''^T per graph: M[u, v] = (count(src=u,dst=v) + I[u,v]) * rd[v]
    srcl = src % NPG
    dstl = dst % NPG
    gid = src // NPG
    flat = gid * (NPG * NPG) + srcl * NPG + dstl
    cnt = np.bincount(flat, minlength=B * NPG * NPG).astype(np.float32)
    cnt = cnt.reshape(B, NPG, NPG)
    idx = np.arange(NPG)
    cnt[:, idx, idx] += 1.0
    cnt *= rd.reshape(B, 1, NPG)
    mT_all = cnt.reshape(B * NPG, NPG)

    c1r = np.asarray(inputs["conv1_w"], np.float32).reshape(C1, D).T.copy()
    c2r = np.asarray(inputs["conv2_w"], np.float32).transpose(1, 2, 0).reshape(
        C1, KW2 * C2).copy()
    ow = np.asarray(inputs["out_w"], np.float32).reshape(C2, NP2, 2).reshape(
        C2, NP2 * 2).copy()

    in_maps = []
    for c in range(n_cores):
        m = {
            "mT": np.ascontiguousarray(mT_all[c * npc:(c + 1) * npc]),
            "node_feat": np.ascontiguousarray(nf[c * npc:(c + 1) * npc]),
            "rdb": rd[c * npc:(c + 1) * npc].reshape(1, npc).copy(),
            "c1r": c1r.astype(bf),
            "c1b": np.asarray(inputs["conv1_b"], np.float32).reshape(C1, 1),
            "c2r": c2r.astype(bf),
            "c2b": np.asarray(inputs["conv2_b"], np.float32).reshape(C2, 1),
            "ow": ow.astype(bf),
            "ob": np.asarray(inputs["out_b"], np.float32).reshape(1, 2).astype(bf),
        }
        for i in range(4):
            m[f"W{i}"] = np.ascontiguousarray(np.asarray(inputs[f"W{i}"], np.float32))
            m[f"b{i}"] = np.asarray(inputs[f"b{i}"], np.float32).reshape(1, LAT[i])
        in_maps.append(m)
    return in_maps


def kernel(**inputs):
    from concourse import bass_utils
    inputs = {k: np.asarray(v) for k, v in inputs.items()}
    nc = _get_nc(B // N_CORES)
    in_maps = make_in_maps(inputs)
    res = bass_utils.run_bass_kernel_spmd(nc, in_maps, core_ids=list(range(N_CORES)))
    return np.concatenate([np.asarray(r["out"], np.float32) for r in res.results],
                          axis=0)


if __name__ == "__main__":
    nc = build_nc(2)
    print("built ok")
